# revision 4
# baseline (speedup 1.0000x reference)
"""RBF kernel regression (Gauss transform) on 8 Trainium2 NeuronCores.

Computes out = K @ alpha where K[b, n] = exp(-||z_b - x_n||^2 / 2),
z: [2048, 64], dataset: [100000, 64], alpha: [100000, 16].

Strategy (sharding_hint): shard dataset/alpha row-wise (N) across 8 cores.
Each core computes partial[f, b] = sum_n alpha[n, f] * G[n, b] with
G = exp(z.x_n - 0.5*||x_n||^2), and the host applies the remaining
exp(-0.5*||z_b||^2) factor, sums partials over cores, and transposes.

Per-core device pipeline (all operands pre-packed/transposed on host):
  for each n-tile (128 rows) and b-half (1024 cols):
    cross = dsT_tile^T @ zT          (TensorE, float32r fast mode)
    G     = exp(cross + bias[n])     (ScalarE, per-partition bias = -0.5*x^2)
    acc  += alpha_tile^T @ G         (TensorE, accumulating PSUM group)
"""

import sys

if "/opt/trn_rl_repo" not in sys.path:
    sys.path.insert(0, "/opt/trn_rl_repo")

import numpy as np

B = 2048  # batch (queries)
D = 64  # feature dim
F = 16  # output dim
NCORES = 8
N_FULL = 100000
NS = N_FULL // NCORES  # 12500 rows per core
NT = 98  # n-tiles of 128 rows (12544 padded)
NTH = NT // 2  # 49 tiles per partition-half
NSP = NT * 128  # 12544
HALF_COLS = NTH * 128  # 6272
BHALF = 1024  # b chunk per PSUM cross tile / ACT instruction
CHUNK_TILES = 7  # dst DMA chunk granularity (7 column-blocks = 896 cols)
N_CHUNKS = NTH // CHUNK_TILES  # 7


def _pack_core_inputs(z, dataset, alpha):
    """Host-side packing: returns (in_maps, w) where w[b] = exp(-0.5*||z_b||^2)."""
    z = np.ascontiguousarray(z, dtype=np.float32)
    dataset = np.ascontiguousarray(dataset, dtype=np.float32)
    alpha = np.ascontiguousarray(alpha, dtype=np.float32)

    zT = z.T  # [64, B]
    zt_packed = np.concatenate([zT, zT], axis=0)  # [128, B] duplicated halves
    z_sq = np.sum(z.astype(np.float64) ** 2, axis=1)
    w = np.exp(-0.5 * z_sq)  # [B], applied on host at the end

    in_maps = []
    for c in range(NCORES):
        ds_c = dataset[c * NS : (c + 1) * NS]
        al_c = alpha[c * NS : (c + 1) * NS]
        dsp = np.zeros((NSP, D), np.float32)
        dsp[:NS] = ds_c
        alp = np.zeros((NSP, F), np.float32)
        alp[:NS] = al_c

        dsT = dsp.T  # [64, NSP]
        dst_packed = np.concatenate(
            [dsT[:, :HALF_COLS], dsT[:, HALF_COLS:]], axis=0
        )  # [128, 6272]
        xsq_packed = np.ascontiguousarray(
            (-0.5 * np.sum(dsp * dsp, axis=1)).reshape(NT, 128).T
        )  # [128, NT]; column k = bias for tile k
        alp_packed = np.ascontiguousarray(
            alp.reshape(NT, 128, F).transpose(1, 0, 2).reshape(128, NT * F)
        )  # [128, NT*F]; partition p, cols k*F:(k+1)*F = alpha[k*128+p]

        in_maps.append(
            {
                "zt": np.ascontiguousarray(zt_packed),
                "dst": np.ascontiguousarray(dst_packed),
                "alp": alp_packed,
                "xsq": xsq_packed,
            }
        )
    return in_maps, w


def build_nc(nt=NT):
    """Build the Bass module. nt can be reduced for simulator smoke tests."""
    import concourse.bass as bass
    import concourse.tile as tile
    from concourse import bacc, mybir

    assert nt % 2 == 0
    nth = nt // 2
    half_cols = nth * 128

    f32 = mybir.dt.float32
    f32r = mybir.dt.float32r

    nc = bacc.Bacc("TRN2", target_bir_lowering=False, debug=False)
    zt_d = nc.dram_tensor("zt", [128, B], f32r, kind="ExternalInput").ap()
    dst_d = nc.dram_tensor("dst", [128, half_cols], f32r, kind="ExternalInput").ap()
    alp_d = nc.dram_tensor("alp", [128, nt * F], f32r, kind="ExternalInput").ap()
    xsq_d = nc.dram_tensor("xsq", [128, nt], f32, kind="ExternalInput").ap()
    out_d = nc.dram_tensor("out", [F, B], f32, kind="ExternalOutput").ap()

    # dst DMA chunking (overlap load with compute)
    chunk_tiles = CHUNK_TILES if nth % CHUNK_TILES == 0 else 1
    n_chunks = nth // chunk_tiles
    chunk_cols = chunk_tiles * 128

    with tile.TileContext(nc) as tc:
        with (
            tc.tile_pool(name="consts", bufs=1) as consts,
            tc.tile_pool(name="g", bufs=3) as gpool,
            tc.tile_pool(name="ps_cross", bufs=2, space="PSUM") as ps_cross,
            tc.tile_pool(name="ps_acc", bufs=1, space="PSUM") as ps_acc,
        ):
            zt_sb = consts.tile([128, B], f32r, tag="zt")
            nc.sync.dma_start(out=zt_sb, in_=zt_d)
            alp_sb = consts.tile([128, nt * F], f32r, tag="alp")
            nc.sync.dma_start(out=alp_sb, in_=alp_d)
            xsq_sb = consts.tile([128, nt], f32, tag="xsq")
            nc.sync.dma_start(out=xsq_sb, in_=xsq_d)
            dst_sb = []
            for j in range(n_chunks):
                t = consts.tile([128, chunk_cols], f32r, tag=f"dst{j}")
                nc.sync.dma_start(
                    out=t, in_=dst_d[:, j * chunk_cols : (j + 1) * chunk_cols]
                )
                dst_sb.append(t)

            # 4 persistent accumulators: (b-half, 512-sub) -> [16, 512] PSUM bank
            acc_ps = [
                ps_acc.tile([F, 512], f32, tag=f"acc{i}", name=f"acc{i}")
                for i in range(4)
            ]

            for p in range(nth):
                chunk = dst_sb[p // chunk_tiles]
                coff = (p % chunk_tiles) * 128
                for h in (0, 1):
                    k = h * nth + p
                    lhs = chunk[h * 64 : (h + 1) * 64, coff : coff + 128]
                    first = p == 0 and h == 0
                    last = p == nth - 1 and h == 1
                    for bh in (0, 1):
                        ps = ps_cross.tile([128, BHALF], f32, tag="cross")
                        for s in (0, 1):
                            nc.tensor.matmul(
                                ps[:, s * 512 : (s + 1) * 512],
                                lhsT=lhs,
                                rhs=zt_sb[
                                    h * 64 : (h + 1) * 64,
                                    bh * BHALF + s * 512 : bh * BHALF + (s + 1) * 512,
                                ],
                                start=True,
                                stop=True,
                            )
                        g = gpool.tile([128, BHALF], f32r, tag="g")
                        nc.scalar.activation(
                            out=g,
                            in_=ps,
                            func=mybir.ActivationFunctionType.Exp,
                            bias=xsq_sb[:, k : k + 1],
                            scale=1.0,
                        )
                        for s in (0, 1):
                            nc.tensor.matmul(
                                acc_ps[bh * 2 + s][:, :],
                                lhsT=alp_sb[:, k * F : (k + 1) * F],
                                rhs=g[:, s * 512 : (s + 1) * 512],
                                start=first,
                                stop=last,
                            )

            out_sb = consts.tile([F, B], f32, tag="out")
            for i in range(4):
                nc.vector.tensor_copy(
                    out=out_sb[:, i * 512 : (i + 1) * 512], in_=acc_ps[i]
                )
            nc.sync.dma_start(out=out_d, in_=out_sb)

    nc.compile()
    return nc


def run_on_cores(in_maps, trace=False, **kwargs):
    from concourse.bass_utils import run_bass_kernel_spmd

    nc = build_nc()
    return run_bass_kernel_spmd(
        nc, in_maps, core_ids=list(range(NCORES)), trace=trace, **kwargs
    )


def kernel(z, dataset, alpha):
    in_maps, w = _pack_core_inputs(z, dataset, alpha)
    res = run_on_cores(in_maps, trace=False)
    total = np.zeros((F, B), np.float64)
    for r in res.results:
        total += r["out"].astype(np.float64)
    total *= w[None, :]
    return np.ascontiguousarray(total.T.astype(np.float32))


# revision 5
# speedup vs baseline: 1.1061x; 1.1061x over previous
"""RBF kernel regression (Gauss transform) on 8 Trainium2 NeuronCores.

Computes out = K @ alpha where K[b, n] = exp(-||z_b - x_n||^2 / 2),
z: [2048, 64], dataset: [100000, 64], alpha: [100000, 16].

Strategy (sharding_hint): shard dataset/alpha row-wise (N) across 8 cores.
Each core computes partial[f, b] = sum_n alpha[n, f] * G[n, b] with
G = exp(z.x_n - 0.5*||x_n||^2), and the host applies the remaining
exp(-0.5*||z_b||^2) factor, sums partials over cores, and transposes.

Per-core device pipeline (all operands pre-packed/transposed on host):
  for each n-tile (128 rows) and b-half (1024 cols):
    cross = dsT_tile^T @ zT          (TensorE, float32r fast mode)
    G     = exp(cross + bias[n])     (ScalarE, per-partition bias = -0.5*x^2)
    acc  += alpha_tile^T @ G         (TensorE, accumulating PSUM group)
"""

import sys

if "/opt/trn_rl_repo" not in sys.path:
    sys.path.insert(0, "/opt/trn_rl_repo")

import numpy as np

B = 2048  # batch (queries)
D = 64  # feature dim
F = 16  # output dim
NCORES = 8
N_FULL = 100000
NS = N_FULL // NCORES  # 12500 rows per core
NT = 98  # n-tiles of 128 rows (12544 padded)
NTH = NT // 2  # 49 tiles per partition-half
NSP = NT * 128  # 12544
HALF_COLS = NTH * 128  # 6272
BHALF = 1024  # b chunk per PSUM cross tile / ACT instruction
CHUNK_TILES = 7  # dst DMA chunk granularity (7 column-blocks = 896 cols)
N_CHUNKS = NTH // CHUNK_TILES  # 7


def _pack_core_inputs(z, dataset, alpha):
    """Host-side packing: returns (in_maps, w) where w[b] = exp(-0.5*||z_b||^2)."""
    z = np.ascontiguousarray(z, dtype=np.float32)
    dataset = np.ascontiguousarray(dataset, dtype=np.float32)
    alpha = np.ascontiguousarray(alpha, dtype=np.float32)

    import ml_dtypes

    zT = z.T  # [64, B]
    zt_packed = np.concatenate([zT, zT], axis=0).astype(np.float16)  # [128, B]
    z_sq = np.sum(z.astype(np.float64) ** 2, axis=1)
    w = np.exp(-0.5 * z_sq)  # [B], applied on host at the end

    in_maps = []
    for c in range(NCORES):
        ds_c = dataset[c * NS : (c + 1) * NS]
        al_c = alpha[c * NS : (c + 1) * NS]
        dsp = np.zeros((NSP, D), np.float32)
        dsp[:NS] = ds_c
        alp = np.zeros((NSP, F), np.float32)
        alp[:NS] = al_c

        dsT = dsp.T  # [64, NSP]
        dst_packed = np.concatenate(
            [dsT[:, :HALF_COLS], dsT[:, HALF_COLS:]], axis=0
        ).astype(np.float16)  # [128, 6272]
        xsq_packed = np.ascontiguousarray(
            (-0.5 * np.sum(dsp * dsp, axis=1)).reshape(NT, 128).T
        )  # [128, NT]; column k = bias for tile k
        alp_packed = np.ascontiguousarray(
            alp.reshape(NT, 128, F).transpose(1, 0, 2).reshape(128, NT * F)
        ).astype(ml_dtypes.bfloat16)  # [128, NT*F]

        in_maps.append(
            {
                "zt": np.ascontiguousarray(zt_packed),
                "dst": np.ascontiguousarray(dst_packed),
                "alp": alp_packed,
                "xsq": xsq_packed,
            }
        )
    return in_maps, w


def build_nc(nt=NT):
    """Build the Bass module. nt can be reduced for simulator smoke tests."""
    import concourse.bass as bass
    import concourse.tile as tile
    from concourse import bacc, mybir

    assert nt % 2 == 0
    nth = nt // 2
    half_cols = nth * 128

    f32 = mybir.dt.float32
    f16 = mybir.dt.float16
    bf16 = mybir.dt.bfloat16

    nc = bacc.Bacc("TRN2", target_bir_lowering=False, debug=False)
    zt_d = nc.dram_tensor("zt", [128, B], f16, kind="ExternalInput").ap()
    dst_d = nc.dram_tensor("dst", [128, half_cols], f16, kind="ExternalInput").ap()
    alp_d = nc.dram_tensor("alp", [128, nt * F], bf16, kind="ExternalInput").ap()
    xsq_d = nc.dram_tensor("xsq", [128, nt], f32, kind="ExternalInput").ap()
    out_d = nc.dram_tensor("out", [F, B], f32, kind="ExternalOutput").ap()

    # dst DMA chunking (overlap load with compute)
    chunk_tiles = CHUNK_TILES if nth % CHUNK_TILES == 0 else 1
    n_chunks = nth // chunk_tiles
    chunk_cols = chunk_tiles * 128

    with tile.TileContext(nc) as tc:
        with (
            tc.tile_pool(name="consts", bufs=1) as consts,
            tc.tile_pool(name="g", bufs=3) as gpool,
            tc.tile_pool(name="ps_cross", bufs=2, space="PSUM") as ps_cross,
            tc.tile_pool(name="ps_acc", bufs=1, space="PSUM") as ps_acc,
        ):
            zt_sb = consts.tile([128, B], f16, tag="zt")
            nc.sync.dma_start(out=zt_sb, in_=zt_d)
            alp_sb = consts.tile([128, nt * F], bf16, tag="alp")
            nc.sync.dma_start(out=alp_sb, in_=alp_d)
            xsq_sb = consts.tile([128, nt], f32, tag="xsq")
            nc.sync.dma_start(out=xsq_sb, in_=xsq_d)
            dst_sb = []
            for j in range(n_chunks):
                t = consts.tile([128, chunk_cols], f16, tag=f"dst{j}")
                nc.sync.dma_start(
                    out=t, in_=dst_d[:, j * chunk_cols : (j + 1) * chunk_cols]
                )
                dst_sb.append(t)

            # 4 persistent accumulators: (b-half, 512-sub) -> [16, 512] PSUM bank
            acc_ps = [
                ps_acc.tile([F, 512], f32, tag=f"acc{i}", name=f"acc{i}")
                for i in range(4)
            ]

            for p in range(nth):
                chunk = dst_sb[p // chunk_tiles]
                coff = (p % chunk_tiles) * 128
                for h in (0, 1):
                    k = h * nth + p
                    lhs = chunk[h * 64 : (h + 1) * 64, coff : coff + 128]
                    first = p == 0 and h == 0
                    last = p == nth - 1 and h == 1
                    for bh in (0, 1):
                        ps = ps_cross.tile([128, BHALF], f32, tag="cross")
                        for s in (0, 1):
                            nc.tensor.matmul(
                                ps[:, s * 512 : (s + 1) * 512],
                                lhsT=lhs,
                                rhs=zt_sb[
                                    h * 64 : (h + 1) * 64,
                                    bh * BHALF + s * 512 : bh * BHALF + (s + 1) * 512,
                                ],
                                start=True,
                                stop=True,
                            )
                        g = gpool.tile([128, BHALF], bf16, tag="g")
                        nc.scalar.activation(
                            out=g,
                            in_=ps,
                            func=mybir.ActivationFunctionType.Exp,
                            bias=xsq_sb[:, k : k + 1],
                            scale=1.0,
                        )
                        for s in (0, 1):
                            nc.tensor.matmul(
                                acc_ps[bh * 2 + s][:, :],
                                lhsT=alp_sb[:, k * F : (k + 1) * F],
                                rhs=g[:, s * 512 : (s + 1) * 512],
                                start=first,
                                stop=last,
                            )

            out_sb = consts.tile([F, B], f32, tag="out")
            for i in range(4):
                nc.vector.tensor_copy(
                    out=out_sb[:, i * 512 : (i + 1) * 512], in_=acc_ps[i]
                )
            nc.sync.dma_start(out=out_d, in_=out_sb)

    nc.compile()
    return nc


def run_on_cores(in_maps, trace=False, **kwargs):
    from concourse.bass_utils import run_bass_kernel_spmd

    nc = build_nc()
    return run_bass_kernel_spmd(
        nc, in_maps, core_ids=list(range(NCORES)), trace=trace, **kwargs
    )


def kernel(z, dataset, alpha):
    in_maps, w = _pack_core_inputs(z, dataset, alpha)
    res = run_on_cores(in_maps, trace=False)
    total = np.zeros((F, B), np.float64)
    for r in res.results:
        total += r["out"].astype(np.float64)
    total *= w[None, :]
    return np.ascontiguousarray(total.T.astype(np.float32))
